# revision 17
# baseline (speedup 1.0000x reference)
"""DPLSTM Trainium2 kernel: T=2048, B=32, D=H=512, 8 NeuronCores.

Strategy (data-parallel, per sharding hint):
  - Batch B=32 sharded 8 ways -> BS=4 per core; LSTM weights replicated.
  - Everything on-chip lives "transposed": gates/hidden on the 128
    partitions, (time x batch) on the free dimension.  The recurrent
    matmul out[128 gate rows, 4 batch] = w_hh_tile.T-as-lhsT @ hT uses
    fp16 stationary weight tiles (fast weight load) with N=4 moving
    columns; PSUM accumulates fp32 over the 4 K-chunks of hidden.
  - gx = x @ w_ih.T + (b_ih + b_hh) is precomputed per 256-step time
    block with fp32r matmuls (full rate at N=512) and stays in SBUF.
  - Elementwise gate math runs on [128, 16] tiles (4 hid-chunks x 4
    batch) - sigmoid/tanh on ScalarE, muls/adds on VectorE.  Each gate
    type (i,f,g,o) gets its own PSUM bank so PE writes and DVE/ACT
    reads never collide on a bank.
  - h_t is written in fp16 directly into the history buffer that both
    feeds the next step's matmul and DMAs out as the block's output.
"""

import numpy as np

import concourse.bass as bass
import concourse.mybir as mybir
import concourse.tile as tile
from concourse import bass_utils
from concourse.bass import ds, ts

P = 128
T, B, D, H = 2048, 32, 512, 512
NCORES = 8
BS = B // NCORES           # 4 batch rows per core
G4 = 4 * H                 # 2048 gate rows
KC = D // P                # 4 contraction chunks
GT = G4 // P               # 16 gate partition-tiles

F32 = mybir.dt.float32
F16 = mybir.dt.float16
F32R = mybir.dt.float32r
AF = mybir.ActivationFunctionType
ALU = mybir.AluOpType
ET = mybir.EngineType

SIG_SLOT = BS * KC         # 16 floats of h/c state per partition


def _split_waits(nc, maxw=1):
    """Workaround: this walrus build only accepts `maxw` sem-wait commands
    per instruction; hoist extras onto preceding same-engine no-ops."""
    nsplit = 0
    for f in nc.m.functions:
        for bb in f.blocks:
            new_insts = []
            for inst in bb.instructions:
                si = getattr(inst, "sync_info", None)
                if si is not None and si.on_wait and len(si.on_wait) > maxw:
                    waits = list(si.on_wait)
                    extra, keep = waits[:-maxw], waits[-maxw:]
                    for i in range(0, len(extra), maxw):
                        d = mybir.InstNoOp(
                            name=f"{inst.name}_wsplit{i}",
                            ins=[],
                            outs=[],
                            sync_info=mybir.SyncInfo(
                                on_wait=extra[i : i + maxw], on_update=[]
                            ),
                        )
                        d.engine = inst.engine
                        nc.register_instruction(d, overwrite=True)
                        new_insts.append(d)
                        nsplit += 1
                    si.on_wait = keep
                new_insts.append(inst)
            bb.instructions[:] = new_insts
    return nsplit


def build_nc(t_total=T, tb=256, unroll=8):
    nblk = t_total // tb
    assert nblk * tb == t_total

    nc = bass.Bass("TRN2", target_bir_lowering=False, debug=False, num_devices=NCORES)
    t_x = nc.dram_tensor("x", [t_total, BS, D], F32, kind="ExternalInput")
    t_h0 = nc.dram_tensor("h0", [BS, H], F32, kind="ExternalInput")
    t_c0 = nc.dram_tensor("c0", [BS, H], F32, kind="ExternalInput")
    t_wih = nc.dram_tensor("w_ih", [G4, D], F32, kind="ExternalInput")
    t_whh = nc.dram_tensor("w_hh", [G4, H], F32, kind="ExternalInput")
    t_bias = nc.dram_tensor("bias", [G4], F32, kind="ExternalInput")
    t_out = nc.dram_tensor("out", [t_total, BS, H], F16, kind="ExternalOutput")
    t_hn = nc.dram_tensor("hn", [BS, H], F16, kind="ExternalOutput")
    t_cn = nc.dram_tensor("cn", [BS, H], F32, kind="ExternalOutput")

    with tile.TileContext(nc) as tc:
        _emit(nc, tc, t_x, t_h0, t_c0, t_wih, t_whh, t_bias, t_out, t_hn, t_cn,
              t_total, tb, nblk, unroll)
    _split_waits(nc)
    return nc


def _emit(nc, tc, t_x, t_h0, t_c0, t_wih, t_whh, t_bias, t_out, t_hn, t_cn,
          t_total, tb, nblk, unroll):
    from contextlib import ExitStack

    with ExitStack() as ctx:
        const = ctx.enter_context(tc.tile_pool(name="const", bufs=1))
        w16 = const.tile([P, KC, G4], F16, tag="w16")      # w_hh.T fp16 tiles
        wih = const.tile([P, KC, G4], F16, tag="wih")      # w_ih.T fp16
        xt16 = const.tile([P, KC, tb * BS], F16, tag="xt16")
        gx = const.tile([P, GT, tb * BS], F32, tag="gx")   # block input proj
        bias_sb = const.tile([P, GT], F32, tag="bias")
        cst = const.tile([P, SIG_SLOT], F32, tag="cst")    # cell state
        h0f = const.tile([P, SIG_SLOT], F32, tag="h0f")

        xpool = ctx.enter_context(tc.tile_pool(name="xp", bufs=2))
        hpool = ctx.enter_context(tc.tile_pool(name="hp", bufs=2))
        work = ctx.enter_context(tc.tile_pool(name="wk", bufs=1))
        ps1 = ctx.enter_context(tc.tile_pool(name="ps1", bufs=2, space="PSUM"))
        psr = ctx.enter_context(tc.tile_pool(name="psr", bufs=1, space="PSUM"))

        # ---- one-time loads ----
        with tc.tile_pool(name="stage", bufs=1) as stage:
            whh_f32 = stage.tile([P, KC, G4], F32, tag="whhs")
            for c in range(KC):
                nc.sync.dma_start(
                    whh_f32[:, c, :],
                    t_whh.ap().rearrange("g (c p) -> p c g", p=P)[:, c, :],
                )
            nc.vector.tensor_copy(w16[:], whh_f32[:])      # cast to fp16
            wih_f32 = stage.tile([P, KC, G4], F32, tag="whhs")
            for c in range(KC):
                nc.sync.dma_start(
                    wih_f32[:, c, :],
                    t_wih.ap().rearrange("g (c p) -> p c g", p=P)[:, c, :],
                )
            nc.vector.tensor_copy(wih[:], wih_f32[:])      # cast to fp16
        nc.sync.dma_start(bias_sb[:], t_bias.ap().rearrange("(g p) -> p g", p=P))
        for c in range(KC):
            nc.sync.dma_start(
                cst[:, ts(c, BS)],
                t_c0.ap().rearrange("b (c p) -> p c b", p=P)[:, c, :],
            )
            nc.sync.dma_start(
                h0f[:, ts(c, BS)],
                t_h0.ap().rearrange("b (c p) -> p c b", p=P)[:, c, :],
            )

        GORDER = [1, 0, 2, 3]  # f, i, g, o  (w_hh row blocks are i,f,g,o)
        hist = None

        for blk in range(nblk):
            xt = xpool.tile([P, KC, tb * BS], F32, tag="xt")
            xr = t_x.ap()[ts(blk, tb)].rearrange("t b (c p) -> p c t b", p=P)
            xtv = xt[:].rearrange("p c (t b) -> p c t b", b=BS)
            for c in range(KC):
                for b in range(BS):
                    nc.sync.dma_start(xtv[:, c, :, b], xr[:, c, :, b])

            prev_hist = hist
            hist = hpool.tile([P, (tb + 1) * SIG_SLOT], F16, tag="hist")

            nc.vector.tensor_copy(xt16[:], xt[:])          # cast to fp16

            # ---- phase 1: gx = x @ w_ih.T + bias for this block ----
            for g in range(GT):
                pg = ps1.tile([P, tb * BS], F32, tag="ps1")
                lim = tb * BS
                for c in range(KC):
                    lhs = wih[:, c, ts(g, P)]
                    for n0 in range(0, lim, 512):
                        nn = min(512, lim - n0)
                        nc.tensor.matmul(
                            pg[:, n0 : n0 + nn],
                            lhs,
                            xt16[:, c, n0 : n0 + nn],
                            start=(c == 0),
                            stop=(c == KC - 1),
                        )
                nc.vector.tensor_scalar_add(gx[:, g, :], pg[:], bias_sb[:, g : g + 1])

            # ---- phase 2: tb recurrence steps, fully unrolled ----
            # hist[:, 0:16] holds h_{t-1} entering the block; step i reads
            # hist slice i and writes slice i+1.  All offsets static.
            if blk == 0:
                nc.vector.tensor_copy(hist[:, 0:SIG_SLOT], h0f[:])
            else:
                nc.vector.tensor_copy(
                    hist[:, 0:SIG_SLOT],
                    prev_hist[:, tb * SIG_SLOT : (tb + 1) * SIG_SLOT],
                )

            for i in range(tb):
                h_prev = hist[:, ts(i, SIG_SLOT)]
                acts = {}
                for G in GORDER:
                    pG = psr.tile([P, SIG_SLOT], F32, tag=f"psr{G}", name=f"psr{G}")
                    for j in range(KC):
                        gidx = 4 * G + j
                        for c in range(KC):
                            nc.tensor.matmul(
                                pG[:, ts(j, BS)],
                                w16[:, c, ts(gidx, P)],
                                h_prev[:, ts(c, BS)],
                                start=(c == 0),
                                stop=(c == KC - 1),
                            )
                    nc.vector.tensor_tensor(
                        out=pG[:].rearrange("p (j b) -> p j b", j=KC),
                        in0=pG[:].rearrange("p (j b) -> p j b", j=KC),
                        in1=gx[:, ts(G, KC), ts(i, BS)],
                        op=ALU.add,
                    )
                    aG = work.tile([P, SIG_SLOT], F32, tag=f"act{G}", name=f"act{G}")
                    acts[G] = aG
                    fn = AF.Tanh if G == 2 else AF.Sigmoid
                    nc.scalar.activation(aG[:], pG[:], fn)

                t1 = work.tile([P, SIG_SLOT], F32, tag="t1", name="t1")
                t2 = work.tile([P, SIG_SLOT], F32, tag="t2", name="t2")
                nc.vector.tensor_mul(t1[:], acts[1][:], cst[:])
                nc.vector.tensor_mul(t2[:], acts[0][:], acts[2][:])
                nc.vector.tensor_add(cst[:], t1[:], t2[:])
                tch = work.tile([P, SIG_SLOT], F32, tag="tch", name="tch")
                nc.scalar.activation(tch[:], cst[:], AF.Tanh)
                nc.vector.tensor_tensor(
                    out=hist[:, ts(i + 1, SIG_SLOT)],
                    in0=acts[3][:],
                    in1=tch[:],
                    op=ALU.mult,
                )

            # ---- block output ----
            outr = t_out.ap()[ts(blk, tb)].rearrange("t b (c p) -> p c t b", p=P)
            hv = hist[:, SIG_SLOT:].rearrange("p (t c b) -> p t c b", t=tb, c=KC)
            for c in range(KC):
                for b in range(BS):
                    nc.sync.dma_start(outr[:, c, :, b], hv[:, :, c, b])

        for c in range(KC):
            nc.sync.dma_start(
                t_hn.ap().rearrange("b (c p) -> p c b", p=P)[:, c, :],
                hist[:, tb * SIG_SLOT + c * BS : tb * SIG_SLOT + (c + 1) * BS],
            )
            nc.sync.dma_start(
                t_cn.ap().rearrange("b (c p) -> p c b", p=P)[:, c, :],
                cst[:, ts(c, BS)],
            )


_NC_CACHE = {}


def _get_nc(key=(T, 256, 8)):
    if key not in _NC_CACHE:
        _NC_CACHE[key] = build_nc(*key)
    return _NC_CACHE[key]


def run_on_cores(nc, x, h0, c0, w_ih, b_ih, w_hh, b_hh, **run_kwargs):
    x = np.ascontiguousarray(np.asarray(x, dtype=np.float32))
    h0 = np.asarray(h0, dtype=np.float32)
    c0 = np.asarray(c0, dtype=np.float32)
    bias = (np.asarray(b_ih, np.float32) + np.asarray(b_hh, np.float32))
    w_ih = np.ascontiguousarray(np.asarray(w_ih, np.float32))
    w_hh = np.ascontiguousarray(np.asarray(w_hh, np.float32))
    in_maps = []
    for i in range(NCORES):
        sl = slice(i * BS, (i + 1) * BS)
        in_maps.append({
            "x": np.ascontiguousarray(x[:, sl]),
            "h0": np.ascontiguousarray(h0[sl]),
            "c0": np.ascontiguousarray(c0[sl]),
            "w_ih": w_ih,
            "w_hh": w_hh,
            "bias": bias,
        })
    res = bass_utils.run_bass_kernel_spmd(
        nc, in_maps, core_ids=list(range(NCORES)), **run_kwargs
    )
    out = np.concatenate(
        [res.results[i]["out"].astype(np.float32) for i in range(NCORES)], axis=1
    )
    hn = np.concatenate(
        [res.results[i]["hn"].astype(np.float32) for i in range(NCORES)], axis=0
    )
    cn = np.concatenate(
        [res.results[i]["cn"].astype(np.float32) for i in range(NCORES)], axis=0
    )
    return (out, hn, cn), res


def kernel(x, h0, c0, w_ih, b_ih, w_hh, b_hh):
    nc = _get_nc()
    (out, hn, cn), _ = run_on_cores(nc, x, h0, c0, w_ih, b_ih, w_hh, b_hh)
    return out, hn, cn
